# revision 8
# baseline (speedup 1.0000x reference)
"""BatchAllTripletLoss v2: label-sorted padded layout, 8-core SPMD.

Labels are sorted on host; class c's members occupy virtual slots
[32c, 32c+32) (pad slots are dummies with zero embeddings and sentinel
labels). Each core owns 4 classes = 128 virtual anchors. An anchor's
positives all lie in the core's own 128 local virtual columns, so the
positive (j) axis collapses from 512 to a [128x128] local block; the
negative (k) axis stays the real 512 columns.

Phase B per 4-anchor group g (32 groups): PE replicates the 4 anchors'
bias rows (0.1 - pd over real k, invalid -> -1e9) into a [128,512] PSUM
tile via an exact fp32 one-hot matmul; ACT computes Relu(B4 + apcol[:,g])
(apcol holds ap'[anchor, j-slot] per partition) with a fused free-axis
sum; DVE counts positives with a fused sum. Host reduces [128,32]
partials in float64.
"""
import os
import sys

for _p in ("/opt/trn_rl_repo",):
    if os.path.isdir(_p) and _p not in sys.path:
        sys.path.insert(0, _p)

import numpy as np

import concourse.bacc as bacc
import concourse.tile as tile
from concourse import mybir
from concourse import bass_utils

N = 512
D = 256
N_CORES = 8
W = 32                      # padded slots per class
NC_CLS = 4                  # classes per core
VA = W * NC_CLS             # 128 virtual anchors per core
NG = VA // 4                # 32 groups of 4 anchors
MARGIN = 0.1
EPS = 1e-16
BIG = 1e9

F32 = mybir.dt.float32
BF16 = mybir.dt.bfloat16
AF = mybir.ActivationFunctionType
OP = mybir.AluOpType

_PROGRAM_CACHE = {}


def build_program(n_rep=1, loop=None, glist=None):
    """loop=None: straight-line graded program. loop="B": For_i(n_rep) around
    phase B. loop="A": For_i(n_rep) around input DMAs + phase A (no phase B).
    glist: tuple of phase-B group indices to compute (others are all-dummy
    and contribute zero); None = all NG groups."""
    if glist is None:
        glist = tuple(range(NG))
    nc = bacc.Bacc(trn_type="TRN2")

    eva_d = nc.dram_tensor("embT_vanch", [128, 2, VA], F32, kind="ExternalInput")
    etr_d = nc.dram_tensor("ET_real", [128, 2, N], F32, kind="ExternalInput")
    labr_d = nc.dram_tensor("lab_real_bc", [128, N], F32, kind="ExternalInput")
    labv_d = nc.dram_tensor("labv_col", [VA, 1], F32, kind="ExternalInput")
    lloc_d = nc.dram_tensor("labloc_bc", [VA, VA], F32, kind="ExternalInput")
    iloc_d = nc.dram_tensor("iotaloc_bc", [VA, VA], F32, kind="ExternalInput")
    vidx_d = nc.dram_tensor("validx_col", [VA, 1], F32, kind="ExternalInput")
    ones_k1_d = nc.dram_tensor("ones_k1", [1, 128], F32, kind="ExternalInput")
    ones_col_d = nc.dram_tensor("ones_col", [128, 1], F32, kind="ExternalInput")
    ident_d = nc.dram_tensor("ident", [128, 128], F32, kind="ExternalInput")
    sel_d = nc.dram_tensor("sel128", [128, NG * 128], F32, kind="ExternalInput")
    rsum_d = nc.dram_tensor("rsum", [128, NG], F32, kind="ExternalOutput")
    csum_d = nc.dram_tensor("csum", [128, NG], F32, kind="ExternalOutput")

    with tile.TileContext(nc) as tc:
        with tc.tile_pool(name="persist", bufs=1) as persist, \
             tc.tile_pool(name="tmpA", bufs=1) as tmpA, \
             tc.tile_pool(name="psA", bufs=3, space="PSUM") as psA, \
             tc.tile_pool(name="psumB", bufs=2, space="PSUM") as psumB, \
             tc.tile_pool(name="rB", bufs=4) as rB, \
             tc.tile_pool(name="gB", bufs=2) as gB:

            eva_sb = persist.tile([128, 2, VA], F32)
            etr_sb = persist.tile([128, 2, N], F32)
            labr_sb = persist.tile([128, N], F32)
            labv_sb = persist.tile([VA, 1], F32)
            lloc_sb = persist.tile([VA, VA], F32)
            iloc_sb = persist.tile([VA, VA], F32)
            vidx_sb = persist.tile([VA, 1], F32)
            ones_k1 = persist.tile([1, 128], F32)
            ones_col = persist.tile([128, 1], F32)
            ident_sb = persist.tile([128, 128], F32)
            sel_sb = persist.tile([128, NG * 128], F32)
            apcol_sb = persist.tile([128, NG], F32)
            bias_sb = persist.tile([VA, N], F32)
            rsum_sb = persist.tile([128, NG], F32)
            csum_sb = persist.tile([128, NG], F32)

            def input_dmas():
                nc.sync.dma_start(eva_sb[:], eva_d.ap()[:])
                nc.sync.dma_start(etr_sb[:], etr_d.ap()[:])
                nc.sync.dma_start(labr_sb[:], labr_d.ap()[:])
                nc.sync.dma_start(labv_sb[:], labv_d.ap()[:])
                nc.sync.dma_start(lloc_sb[:], lloc_d.ap()[:])
                nc.sync.dma_start(iloc_sb[:], iloc_d.ap()[:])
                nc.sync.dma_start(vidx_sb[:], vidx_d.ap()[:])
                nc.sync.dma_start(ones_k1[:], ones_k1_d.ap()[:])
                nc.sync.dma_start(ones_col[:], ones_col_d.ap()[:])
                nc.sync.dma_start(ident_sb[:], ident_d.ap()[:])
                # split the big selector across chunks (queue parallelism)
                for i in range(4):
                    sl = slice(i * NG * 32, (i + 1) * NG * 32)
                    nc.sync.dma_start(sel_sb[:, sl], sel_d.ap()[:, sl])

            def phase_a():
                # --- sq_anchor (per virtual anchor) ---
                sqe = tmpA.tile([128, 2, VA], F32)
                nc.vector.tensor_tensor(sqe[:], eva_sb[:], eva_sb[:], op=OP.mult)
                sqa_ps = psA.tile([1, 2, VA], F32, tag="m")
                nc.tensor.matmul(sqa_ps[:], lhsT=ones_col[:], rhs=sqe[:],
                                 start=True, stop=True)
                sqa2 = tmpA.tile([1, 2, VA], F32)
                nc.scalar.copy(sqa2[:], sqa_ps[:])
                sqa_row = tmpA.tile([1, VA], F32)
                nc.vector.tensor_tensor(sqa_row[:], sqa2[0:1, 0, :],
                                        sqa2[0:1, 1, :], op=OP.add)
                sqat_ps = psA.tile([VA, 1], F32, tag="m")
                nc.tensor.transpose(sqat_ps[:], sqa_row[:], ident_sb[0:1, 0:1])
                sqa_col = persist.tile([VA, 1], F32)
                nc.scalar.copy(sqa_col[:], sqat_ps[:])

                # --- local pd block (positives side), [VA, VA] ---
                gl_ps = psA.tile([VA, VA], F32, tag="m")
                for h in range(2):
                    nc.tensor.matmul(gl_ps[:], lhsT=eva_sb[:, h, :],
                                     rhs=eva_sb[:, h, :],
                                     start=(h == 0), stop=(h == 1))
                d1l = tmpA.tile([VA, VA], F32)
                nc.scalar.activation(d1l[:], gl_ps[:], AF.Identity,
                                     bias=sqa_col[:], scale=-2.0)
                sqlbc_ps = psA.tile([VA, VA], F32, tag="m")
                nc.tensor.matmul(sqlbc_ps[:], lhsT=ones_k1[:], rhs=sqa_row[:],
                                 start=True, stop=True)
                d2l = tmpA.tile([VA, VA], F32)
                nc.vector.tensor_tensor(d2l[:], d1l[:], sqlbc_ps[:], op=OP.add)
                drl = tmpA.tile([VA, VA], F32)
                nc.vector.tensor_scalar(drl[:], d2l[:], 0.0, None, op0=OP.max)
                pdl = tmpA.tile([VA, VA], F32)
                nc.scalar.activation(pdl[:], drl[:], AF.Sqrt)
                # ap' = pd*maskP + (maskP-1)*BIG over local columns
                eql = tmpA.tile([VA, VA], F32)
                nc.vector.tensor_scalar(eql[:], lloc_sb[:], labv_sb[:], None,
                                        op0=OP.is_equal)
                ohl = tmpA.tile([VA, VA], F32)
                nc.vector.tensor_scalar(ohl[:], iloc_sb[:], vidx_sb[:], None,
                                        op0=OP.is_equal)
                mP = tmpA.tile([VA, VA], F32)
                nc.vector.tensor_tensor(mP[:], eql[:], ohl[:], op=OP.subtract)
                auxP = tmpA.tile([VA, VA], F32)
                nc.vector.tensor_scalar(auxP[:], mP[:], BIG, -BIG, op0=OP.mult,
                                        op1=OP.add)
                pdP = tmpA.tile([VA, VA], F32)
                nc.vector.tensor_tensor(pdP[:], pdl[:], mP[:], op=OP.mult)
                apl = tmpA.tile([VA, VA], F32)
                nc.vector.tensor_tensor(apl[:], pdP[:], auxP[:], op=OP.add)
                # transpose and shuffle into per-group columns:
                # apcol[32q+w, 8gc+gl] = apT[32gc+w, 32gc+4gl+q]
                aplT_ps = psA.tile([VA, VA], F32, tag="m")
                nc.tensor.transpose(aplT_ps[:], apl[:], ident_sb[:])
                aplT = tmpA.tile([VA, VA], F32)
                nc.scalar.copy(aplT[:], aplT_ps[:])
                for gc in range(NC_CLS):
                    blk = aplT[32 * gc:32 * gc + 32, 32 * gc:32 * gc + 32]
                    blk = blk.rearrange("p (gl q) -> p gl q", q=4)
                    for q in range(4):
                        nc.sync.dma_start(
                            apcol_sb[32 * q:32 * q + 32, 8 * gc:8 * gc + 8],
                            blk[:, :, q],
                        )

                # --- real-k pd rows (negatives side), [VA, N] ---
                sq2 = tmpA.tile([128, 2, N], F32)
                nc.vector.tensor_tensor(sq2[:], etr_sb[:], etr_sb[:], op=OP.mult)
                sqf_ps = psA.tile([1, N], F32, tag="m")
                for h in range(2):
                    nc.tensor.matmul(sqf_ps[:], lhsT=ones_col[:], rhs=sq2[:, h, :],
                                     start=(h == 0), stop=(h == 1))
                sqf_sb = tmpA.tile([1, N], F32)
                nc.scalar.copy(sqf_sb[:], sqf_ps[:])

                gr_ps = psA.tile([VA, N], F32, tag="m")
                for h in range(2):
                    nc.tensor.matmul(gr_ps[:], lhsT=eva_sb[:, h, :],
                                     rhs=etr_sb[:, h, :],
                                     start=(h == 0), stop=(h == 1))
                d1r = tmpA.tile([VA, N], F32)
                nc.scalar.activation(d1r[:], gr_ps[:], AF.Identity,
                                     bias=sqa_col[:], scale=-2.0)
                sqfbc_ps = psA.tile([128, N], F32, tag="m")
                nc.tensor.matmul(sqfbc_ps[:], lhsT=ones_k1[:], rhs=sqf_sb[:],
                                 start=True, stop=True)
                d2r = tmpA.tile([VA, N], F32)
                nc.vector.tensor_tensor(d2r[:], d1r[:], sqfbc_ps[:], op=OP.add)
                drr = tmpA.tile([VA, N], F32)
                nc.vector.tensor_scalar(drr[:], d2r[:], 0.0, None, op0=OP.max)
                pdr = tmpA.tile([VA, N], F32)
                nc.scalar.activation(pdr[:], drr[:], AF.Sqrt)
                eqr = tmpA.tile([VA, N], F32)
                nc.vector.tensor_scalar(eqr[:], labr_sb[:], labv_sb[:], None,
                                        op0=OP.is_equal)
                mN = tmpA.tile([VA, N], F32)
                nc.vector.tensor_scalar(mN[:], eqr[:], -1.0, 1.0, op0=OP.mult,
                                        op1=OP.add)
                t2 = tmpA.tile([VA, N], F32)
                nc.vector.tensor_scalar(t2[:], pdr[:], -1.0, MARGIN, op0=OP.mult,
                                        op1=OP.add)
                nm = tmpA.tile([VA, N], F32)
                nc.vector.tensor_tensor(nm[:], t2[:], mN[:], op=OP.mult)
                auxN = tmpA.tile([VA, N], F32)
                nc.vector.tensor_scalar(auxN[:], mN[:], BIG, -BIG, op0=OP.mult,
                                        op1=OP.add)
                nc.vector.tensor_tensor(bias_sb[:], nm[:], auxN[:], op=OP.add)

            def phase_b():
                for g in glist:
                    b4 = psumB.tile([128, N], F32, tag="b4")
                    nc.tensor.matmul(b4[:], lhsT=sel_sb[:, g * 128:(g + 1) * 128],
                                     rhs=bias_sb[:], start=True, stop=True)
                    R = rB.tile([128, N], BF16, tag="R")
                    nc.scalar.activation(
                        R[:], b4[:], AF.Relu,
                        bias=apcol_sb[:, g:g + 1], scale=1.0,
                        accum_out=rsum_sb[:, g:g + 1],
                    )
                    G = gB.tile([128, N], BF16, tag="G")
                    nc.vector.tensor_scalar(
                        G[:], R[:], 0.0, None, op0=OP.is_gt, op1=OP.add,
                        accum_out=csum_sb[:, g:g + 1],
                    )

            nc.vector.memset(rsum_sb[:], 0.0)
            nc.vector.memset(csum_sb[:], 0.0)
            if loop is None:
                input_dmas()
                phase_a()
                phase_b()
            elif loop == "B":
                input_dmas()
                phase_a()
                with tc.For_i(0, n_rep, 1):
                    phase_b()
            elif loop == "A":
                with tc.For_i(0, n_rep, 1):
                    input_dmas()
                    phase_a()
            else:
                raise ValueError(loop)

            nc.sync.dma_start(rsum_d.ap()[:], rsum_sb[:])
            nc.sync.dma_start(csum_d.ap()[:], csum_sb[:])

    nc.compile()
    return nc


def get_program(n_rep=1, loop=None, glist=None):
    key = (n_rep, loop, glist)
    if key not in _PROGRAM_CACHE:
        _PROGRAM_CACHE[key] = build_program(n_rep, loop, glist)
    return _PROGRAM_CACHE[key]


def host_layout(embeddings, labels):
    """Sort by label into padded virtual slots. Classes are dealt to
    (core, position) by descending size so that per-position group counts
    (set by the largest class at that position) are minimized. Returns
    per-core host arrays + the phase-B group list."""
    emb = np.ascontiguousarray(np.asarray(embeddings, dtype=np.float32))
    lab = np.asarray(labels).astype(np.int64)
    assert emb.shape == (N, D)
    perm = np.argsort(lab, kind="stable")
    emb_p = emb[perm]
    lab_p = lab[perm].astype(np.float32)
    counts = np.bincount(lab, minlength=32)
    assert counts.max() <= W, f"class too large for W={W}: {counts.max()}"
    starts = np.zeros(33, dtype=np.int64)
    starts[1:] = np.cumsum(counts)
    order = np.argsort(-counts, kind="stable")      # class ids, biggest first
    n_virt = N_CORES * VA  # 1024
    v_emb = np.zeros((n_virt, D), dtype=np.float32)
    v_lab_j = np.full(n_virt, -1.0, dtype=np.float32)   # j-side sentinel
    v_lab_a = np.full(n_virt, -2.0, dtype=np.float32)   # anchor-side sentinel
    glist = []
    for cc in range(NC_CLS):                        # position within core
        chunk = order[cc * N_CORES:(cc + 1) * N_CORES]
        for gl in range((counts[chunk].max() + 3) // 4):
            glist.append(8 * cc + gl)
        for core, cls in enumerate(chunk):
            m = counts[cls]
            base = core * VA + cc * W
            if m:
                v_emb[base:base + m] = emb_p[starts[cls]:starts[cls] + m]
                v_lab_j[base:base + m] = cls
                v_lab_a[base:base + m] = cls
    return emb_p, lab_p, v_emb, v_lab_j, v_lab_a, tuple(sorted(glist))


def make_in_maps(embeddings, labels):
    emb_p, lab_p, v_emb, v_lab_j, v_lab_a, glist = host_layout(embeddings, labels)
    ET_real = np.ascontiguousarray(
        emb_p.T.reshape(2, 128, N).transpose(1, 0, 2))          # [128, 2, 512]
    lab_real_bc = np.ascontiguousarray(np.broadcast_to(lab_p[None, :], (128, N)))
    ones_k1 = np.ones((1, 128), dtype=np.float32)
    ones_col = np.ones((128, 1), dtype=np.float32)
    ident = np.eye(128, dtype=np.float32)
    iotaloc_bc = np.ascontiguousarray(
        np.broadcast_to(np.arange(VA, dtype=np.float32)[None, :], (VA, VA)))
    vidx_col = np.arange(VA, dtype=np.float32)[:, None]
    # sel128[r, g*128 + p] = 1 iff r == 4g + p//32
    sel = np.zeros((128, NG, 128), dtype=np.float32)
    for g in range(NG):
        for q in range(4):
            sel[4 * g + q, g, 32 * q:32 * q + 32] = 1.0
    sel = np.ascontiguousarray(sel.reshape(128, NG * 128))
    in_maps = []
    for c in range(N_CORES):
        lo = c * VA
        sl = v_emb[lo:lo + VA]                                  # [128, 256]
        eva = np.ascontiguousarray(sl.T.reshape(2, 128, VA).transpose(1, 0, 2))
        in_maps.append({
            "embT_vanch": eva,
            "ET_real": ET_real,
            "lab_real_bc": lab_real_bc,
            "labv_col": np.ascontiguousarray(v_lab_a[lo:lo + VA, None]),
            "labloc_bc": np.ascontiguousarray(
                np.broadcast_to(v_lab_j[None, lo:lo + VA], (VA, VA))),
            "iotaloc_bc": iotaloc_bc,
            "validx_col": vidx_col,
            "ones_k1": ones_k1,
            "ones_col": ones_col,
            "ident": ident,
            "sel128": sel,
        })
    return in_maps, glist


def reduce_outputs(results):
    loss_sum = 0.0
    hard_sum = 0.0
    for r in results:
        loss_sum += r["rsum"].astype(np.float64).sum()
        hard_sum += r["csum"].astype(np.float64).sum()
    num_hard = np.float32(hard_sum)
    loss = np.float32(np.float32(loss_sum) / (num_hard + np.float32(EPS)))
    return loss, num_hard


def kernel(embeddings, labels):
    in_maps, glist = make_in_maps(embeddings, labels)
    nc = get_program(glist=glist)
    res = bass_utils.run_bass_kernel_spmd(nc, in_maps, core_ids=list(range(N_CORES)))
    return reduce_outputs(res.results)
